# revision 36
# baseline (speedup 1.0000x reference)
"""Correlation cost volume kernel for Trainium2 (8 NeuronCores, batch-parallel).

cost[b, i, h, x] = mean_c left[b,c,h,x] * right[b,c,h,x-i], i in [0,48), zero for x < i.

Per core (one batch element):
  Inputs are host-cast to bf16, left pre-scaled by 1/128 (exact power of two),
  so no on-device scaling is needed and all DMA traffic is halved.
  For each h row and x-chunk (M=128/128/64): PSUM G[a, j] = sum_c
  lscaled[c, X0+a] * right[c, X0-47+j]. Right is loaded contiguously with
  slack; out-of-range columns read garbage that only reaches the x < i
  triangle, which the host masks to zero.
  PSUM tile [128, 1024] (2 banks; chunk slots at {0,256,512} so no matmul
  crosses a bank). Two DVE/ACT copies per h row cast to bf16 into the group
  rect tile, hl-major row blocks of 464 (A 176 | B 176 | C 112).
  Shear band[a, (hl*3+ci)*48 + k] = G[a, a+k]:
   - scatter groups: one gpsimd local_scatter per group (per-partition
     indices; invalid lanes zeroed) + one full-rate contiguous store.
     hl-major makes a 4-row group's index table a prefix of the full one,
     so the first/last groups are 4-row halves (earlier Pool start/finish).
   - dump groups: 10 quad-block DMAs store the 80-wide diagonal quarters;
     the host extracts the diagonals (no Pool time).
  Shear-store DMAs are emitted a few groups late on nc.sync so their waits
  are satisfied at issue time and never stall a sequencer.
  Host untangles layouts -> (i=47-k, h, x), flips i, zeroes x < i.
"""
import os

import numpy as np
import ml_dtypes

import concourse.bacc as bacc
import concourse.mybir as mybir
import concourse.tile as tile
from concourse.ap import AP
from concourse.bass_utils import run_bass_kernel_spmd

B, C, H, W = 8, 128, 96, 320
D = 48  # disparities
HW = H * W
CHUNKS = [(0, 128, 176), (128, 128, 176), (256, 64, 112)]  # (X0, M, NMM)
ABW = 352  # AB row block: A 176 | B 176 (scatter region)
CW_ = 112  # C row block width (host-shear region)
CBASE = 8 * ABW  # C region offset inside rect (2816)
RECW = 8 * ABW + 8 * CW_  # 3712
RW = 47 + 8 * W + 48  # right tile width incl. slack (2655)
# (h0, nrows, kind): 'S' = gpsimd local_scatter shear, 'D' = quad dump (host shear)
GROUPS = [(0, 4), (4, 4), (8, 8), (16, 8), (24, 8), (32, 8), (40, 8),
          (48, 8), (56, 8), (64, 8), (72, 8), (80, 8), (88, 4), (92, 4)]
OUT2_OFF = np.cumsum([0] + [128 * nr * 96 for h0, nr in GROUPS]).tolist()
QB = 32 * 8 * 80  # one C quad block
QTOT = 2 * QB  # per group (quads 0,1 cover partitions a<64)
STORE_LAG = 7

_cache = {}


def make_idxs():
    """idx[a, hl*352 + ci*176 + col] = (hl*2+ci)*48 + (col-a), col in [a, a+48)."""
    idx = np.full((128, CBASE), -1, dtype=np.int16)
    a = np.arange(128)
    for hl in range(8):
        for ci in range(2):
            s = hl * 2 + ci
            for k in range(D):
                idx[a, hl * ABW + ci * 176 + a + k] = s * D + k
    return idx


def _emit_store(nc, out2, quads, item):
    kind, gi, tile_ = item
    h0, nrows = GROUPS[gi]
    if kind == "band":
        dst = AP(out2.tensor, out2.offset + OUT2_OFF[gi],
                 [[nrows * 96, 128], [1, nrows * 96]])
        nc.sync.dma_start(out=dst, in_=tile_[:, : nrows * 96])
        return
    rp = tile_.ap[0][0]
    qbase = quads.offset + gi * QTOT
    for q in range(2):  # C quads: rows [32q,32q+32), cols [32q,32q+80)
        src = AP(tile_.tensor, tile_.offset + 32 * q * rp + CBASE + 32 * q,
                 [[rp, 32], [CW_, 8], [1, 80]])
        dst = AP(quads.tensor, qbase + q * QB, [[8 * 80, 32], [80, 8], [1, 80]])
        nc.sync.dma_start(out=dst, in_=src)


def _build():
    nc = bacc.Bacc("TRN2", target_bir_lowering=False, debug=False, num_devices=8)
    left = nc.dram_tensor("left", [C, HW], mybir.dt.bfloat16, kind="ExternalInput").ap()
    right = nc.dram_tensor("right", [C, HW], mybir.dt.bfloat16, kind="ExternalInput").ap()
    idxs_in = nc.dram_tensor("idxs", [128, CBASE], mybir.dt.int16, kind="ExternalInput").ap()
    out2 = nc.dram_tensor("out2", [OUT2_OFF[-1]], mybir.dt.bfloat16,
                          kind="ExternalOutput").ap()
    quads = nc.dram_tensor("quads", [len(GROUPS) * QTOT], mybir.dt.bfloat16,
                           kind="ExternalOutput").ap()

    with tile.TileContext(nc) as tc:
        with (
            tc.tile_pool(name="io", bufs=8) as io_pool,
            tc.tile_pool(name="rect", bufs=8) as rect_pool,
            tc.tile_pool(name="band", bufs=8) as band_pool,
            tc.tile_pool(name="const", bufs=1) as const_pool,
            tc.tile_pool(name="ps", bufs=4, space="PSUM") as ps_pool,
        ):
            idx_t = const_pool.tile([128, CBASE], mybir.dt.int16)
            pending = []
            pending_q = []

            for gi, (h0, nrows) in enumerate(GROUPS):
                l_t = io_pool.tile([C, 8 * W], mybir.dt.bfloat16, tag="lt")
                r_t = io_pool.tile([C, RW], mybir.dt.bfloat16, tag="rt")
                nc.sync.dma_start(out=l_t[:, : nrows * W],
                                  in_=left[:, h0 * W : (h0 + nrows) * W])
                nc.sync.dma_start(out=r_t[:, 47 : 47 + nrows * W],
                                  in_=right[:, h0 * W : (h0 + nrows) * W])
                # idx table loaded in prefix pieces so early small groups
                # can scatter before the whole table arrives
                if gi == 0:
                    nc.sync.dma_start(out=idx_t[:, : 4 * ABW],
                                      in_=idxs_in[:, : 4 * ABW])
                elif gi == 1:
                    nc.sync.dma_start(out=idx_t[:, 4 * ABW :],
                                      in_=idxs_in[:, 4 * ABW :])

                rect_g = rect_pool.tile([128, RECW], mybir.dt.bfloat16, tag="rect")
                rp = rect_g.ap[0][0]
                for hl in range(nrows):
                    # 2 PSUM banks; chunk slots at {0,256,512}: no bank crossing.
                    g_ps = ps_pool.tile([128, 1024], mybir.dt.float32, tag="gps")
                    pp = g_ps.ap[0][0]
                    for ci, (X0, M, NMM) in enumerate(CHUNKS):
                        nc.tensor.matmul(
                            g_ps[:M, ci * 256 : ci * 256 + NMM],
                            l_t[:, hl * W + X0 : hl * W + X0 + M],
                            r_t[:, hl * W + X0 : hl * W + X0 + NMM],
                            start=True, stop=True,
                        )
                    dst_ab = AP(rect_g.tensor, rect_g.offset + hl * ABW,
                                [[rp, 128], [176, 2], [1, 176]])
                    src_ab = AP(g_ps.tensor, g_ps.offset, [[pp, 128], [256, 2], [1, 176]])
                    dst_c = rect_g[:, CBASE + hl * CW_ : CBASE + (hl + 1) * CW_]
                    src_c = g_ps[:, 512 : 512 + 112]
                    if hl % 2 == 0:
                        nc.vector.tensor_copy(dst_ab, src_ab)
                        nc.scalar.copy(dst_c, src_c)
                    else:
                        nc.scalar.copy(dst_ab, src_ab)
                        nc.vector.tensor_copy(dst_c, src_c)

                band_g = band_pool.tile([128, 8 * 96], mybir.dt.bfloat16, tag="band")
                nc.gpsimd.local_scatter(
                    band_g[:, : nrows * 96], rect_g[:, : nrows * ABW],
                    idx_t[:, : nrows * ABW],
                    channels=128, num_elems=nrows * 96, num_idxs=nrows * ABW,
                )
                pending.append(("band", gi, band_g))
                pending_q.append(("dump", gi, rect_g))
                while pending_q and pending_q[0][1] <= gi - 5:
                    _emit_store(nc, out2, quads, pending_q.pop(0))
                while pending and pending[0][1] <= gi - STORE_LAG:
                    _emit_store(nc, out2, quads, pending.pop(0))
            for item in pending_q + pending:
                _emit_store(nc, out2, quads, item)
    nc.compile()
    return nc


def _get_nc(_mode=None):
    if "nc" not in _cache:
        _cache["nc"] = _build()
    return _cache["nc"]


def kernel(left_feature, right_feature):
    left_feature = np.asarray(left_feature, dtype=np.float32)
    right_feature = np.asarray(right_feature, dtype=np.float32)
    b, c, h, w = left_feature.shape
    assert (b, c, h, w) == (B, C, H, W)
    nc = _get_nc()
    idx = make_idxs()
    in_maps = []
    for i in range(B):
        lf = (left_feature[i].reshape(C, HW) * np.float32(1.0 / C)).astype(ml_dtypes.bfloat16)
        rf = right_feature[i].reshape(C, HW).astype(ml_dtypes.bfloat16)
        in_maps.append({
            "left": np.ascontiguousarray(lf),
            "right": np.ascontiguousarray(rf),
            "idxs": idx,
        })
    trace = bool(os.environ.get("KERNEL_TRACE"))
    res = run_bass_kernel_spmd(nc, in_maps, core_ids=list(range(B)), trace=trace)
    if trace:
        print("HW exec time:", res.exec_time_ns, "ns")
    outs = []
    a32 = np.arange(32)
    for i in range(B):
        vol = np.empty((D, H, W), dtype=np.float32)
        raw = np.asarray(res.results[i]["out2"]).astype(np.float32)
        for gi, (h0, nr) in enumerate(GROUPS):
            band = raw[OUT2_OFF[gi] : OUT2_OFF[gi + 1]].reshape(128, nr, 2, D)
            v = band.transpose(3, 1, 2, 0)  # [k, hl, ci, a]
            vol[:, h0 : h0 + nr, :256] = v.reshape(D, nr, 256)
        qall = np.asarray(res.results[i]["quads"]).astype(np.float32).reshape(
            len(GROUPS), 2, 32, 8, 80)  # [g, quad, a32, hl, col80]
        for gi, (h0, nr) in enumerate(GROUPS):
            for k in range(D):
                sel = (a32 + k)[None, :, None, None]
                blk = np.take_along_axis(qall[gi], sel, axis=3)[:, :, :, 0]  # [2, 32, 8]
                vol[k, h0 : h0 + nr, 256:] = blk.reshape(64, 8)[:, :nr].T
        outs.append(vol[::-1])  # k = 47 - i
    out = np.stack(outs, axis=0)
    for i in range(1, D):
        out[:, i, :, :i] = 0.0
    return out


if __name__ == "__main__":
    rng = np.random.default_rng(0)
    lf = rng.standard_normal((B, C, H, W), dtype=np.float32)
    rf = rng.standard_normal((B, C, H, W), dtype=np.float32)
    got = kernel(lf, rf)
    for (bb, i, hh, xx) in [(0, 0, 0, 0), (0, 5, 10, 100), (1, 47, 95, 319), (2, 47, 3, 10),
                            (3, 20, 85, 200), (7, 1, 90, 300), (5, 30, 35, 128)]:
        want = float(np.dot(lf[bb, :, hh, xx], rf[bb, :, hh, xx - i]) / C) if xx >= i else 0.0
        print((bb, i, hh, xx), "got", got[bb, i, hh, xx], "want", want)


# revision 38
# speedup vs baseline: 1.0033x; 1.0033x over previous
"""Correlation cost volume kernel for Trainium2 (8 NeuronCores, batch-parallel).

cost[b, i, h, x] = mean_c left[b,c,h,x] * right[b,c,h,x-i], i in [0,48), zero for x < i.

Per core (one batch element):
  Inputs are host-cast to bf16, left pre-scaled by 1/128 (exact power of two),
  so no on-device scaling is needed and all DMA traffic is halved.
  For each h row and x-chunk (M=128/128/64): PSUM G[a, j] = sum_c
  lscaled[c, X0+a] * right[c, X0-47+j]. Right is loaded contiguously with
  slack; out-of-range columns read garbage that only reaches the x < i
  triangle, which the host masks to zero.
  PSUM tile [128, 1024] (2 banks; chunk slots at {0,256,512} so no matmul
  crosses a bank). Two DVE/ACT copies per h row cast to bf16 into the group
  rect tile, hl-major row blocks of 464 (A 176 | B 176 | C 112).
  Shear band[a, (hl*3+ci)*48 + k] = G[a, a+k]:
   - scatter groups: one gpsimd local_scatter per group (per-partition
     indices; invalid lanes zeroed) + one full-rate contiguous store.
     hl-major makes a 4-row group's index table a prefix of the full one,
     so the first/last groups are 4-row halves (earlier Pool start/finish).
   - dump groups: 10 quad-block DMAs store the 80-wide diagonal quarters;
     the host extracts the diagonals (no Pool time).
  Shear-store DMAs are emitted a few groups late on nc.sync so their waits
  are satisfied at issue time and never stall a sequencer.
  Host untangles layouts -> (i=47-k, h, x), flips i, zeroes x < i.
"""
import os

import numpy as np
import ml_dtypes

import concourse.bacc as bacc
import concourse.mybir as mybir
import concourse.tile as tile
from concourse.ap import AP
from concourse.bass_utils import run_bass_kernel_spmd

B, C, H, W = 8, 128, 96, 320
D = 48  # disparities
HW = H * W
CHUNKS = [(0, 128, 176), (128, 128, 176), (256, 64, 112)]  # (X0, M, NMM)
ABW = 352  # AB row block: A 176 | B 176 (scatter region)
CW_ = 112  # C row block width (host-shear region)
CBASE = 8 * ABW  # C region offset inside rect (2816)
RECW = 8 * ABW + 8 * CW_  # 3712
RW = 47 + 8 * W + 48  # right tile width incl. slack (2655)
# (h0, nrows, kind): 'S' = gpsimd local_scatter shear, 'D' = quad dump (host shear)
GROUPS = [(0, 4), (4, 4), (8, 4), (12, 4), (16, 8), (24, 8), (32, 8), (40, 8),
          (48, 8), (56, 8), (64, 8), (72, 8), (80, 8), (88, 4), (92, 4)]
OUT2_OFF = np.cumsum([0] + [128 * nr * 96 for h0, nr in GROUPS]).tolist()
QB = 32 * 8 * 80  # one C quad block
QTOT = 2 * QB  # per group (quads 0,1 cover partitions a<64)
STORE_LAG = 7

_cache = {}


def make_idxs():
    """idx[a, hl*352 + ci*176 + col] = (hl*2+ci)*48 + (col-a), col in [a, a+48)."""
    idx = np.full((128, CBASE), -1, dtype=np.int16)
    a = np.arange(128)
    for hl in range(8):
        for ci in range(2):
            s = hl * 2 + ci
            for k in range(D):
                idx[a, hl * ABW + ci * 176 + a + k] = s * D + k
    return idx


def _emit_store(nc, out2, quads, item):
    kind, gi, tile_ = item
    h0, nrows = GROUPS[gi]
    if kind == "band":
        dst = AP(out2.tensor, out2.offset + OUT2_OFF[gi],
                 [[nrows * 96, 128], [1, nrows * 96]])
        nc.sync.dma_start(out=dst, in_=tile_[:, : nrows * 96])
        return
    rp = tile_.ap[0][0]
    qbase = quads.offset + gi * QTOT
    for q in range(2):  # C quads: rows [32q,32q+32), cols [32q,32q+80)
        src = AP(tile_.tensor, tile_.offset + 32 * q * rp + CBASE + 32 * q,
                 [[rp, 32], [CW_, 8], [1, 80]])
        dst = AP(quads.tensor, qbase + q * QB, [[8 * 80, 32], [80, 8], [1, 80]])
        nc.sync.dma_start(out=dst, in_=src)


def _build():
    nc = bacc.Bacc("TRN2", target_bir_lowering=False, debug=False, num_devices=8)
    left = nc.dram_tensor("left", [C, HW], mybir.dt.bfloat16, kind="ExternalInput").ap()
    right = nc.dram_tensor("right", [C, HW], mybir.dt.bfloat16, kind="ExternalInput").ap()
    idxs_in = nc.dram_tensor("idxs", [128, CBASE], mybir.dt.int16, kind="ExternalInput").ap()
    out2 = nc.dram_tensor("out2", [OUT2_OFF[-1]], mybir.dt.bfloat16,
                          kind="ExternalOutput").ap()
    quads = nc.dram_tensor("quads", [len(GROUPS) * QTOT], mybir.dt.bfloat16,
                           kind="ExternalOutput").ap()

    with tile.TileContext(nc) as tc:
        with (
            tc.tile_pool(name="io", bufs=8) as io_pool,
            tc.tile_pool(name="rect", bufs=8) as rect_pool,
            tc.tile_pool(name="band", bufs=8) as band_pool,
            tc.tile_pool(name="const", bufs=1) as const_pool,
            tc.tile_pool(name="ps", bufs=4, space="PSUM") as ps_pool,
        ):
            idx_t = const_pool.tile([128, CBASE], mybir.dt.int16)
            pending = []
            pending_q = []

            for gi, (h0, nrows) in enumerate(GROUPS):
                l_t = io_pool.tile([C, 8 * W], mybir.dt.bfloat16, tag="lt")
                r_t = io_pool.tile([C, RW], mybir.dt.bfloat16, tag="rt")
                nc.sync.dma_start(out=l_t[:, : nrows * W],
                                  in_=left[:, h0 * W : (h0 + nrows) * W])
                nc.sync.dma_start(out=r_t[:, 47 : 47 + nrows * W],
                                  in_=right[:, h0 * W : (h0 + nrows) * W])
                # idx table loaded in prefix pieces so early small groups
                # can scatter before the whole table arrives
                if gi == 0:
                    nc.sync.dma_start(out=idx_t[:, : 4 * ABW],
                                      in_=idxs_in[:, : 4 * ABW])
                elif gi == 1:
                    nc.sync.dma_start(out=idx_t[:, 4 * ABW :],
                                      in_=idxs_in[:, 4 * ABW :])

                rect_g = rect_pool.tile([128, RECW], mybir.dt.bfloat16, tag="rect")
                rp = rect_g.ap[0][0]
                for hl in range(nrows):
                    # 2 PSUM banks; chunk slots at {0,256,512}: no bank crossing.
                    g_ps = ps_pool.tile([128, 1024], mybir.dt.float32, tag="gps")
                    pp = g_ps.ap[0][0]
                    for ci, (X0, M, NMM) in enumerate(CHUNKS):
                        nc.tensor.matmul(
                            g_ps[:M, ci * 256 : ci * 256 + NMM],
                            l_t[:, hl * W + X0 : hl * W + X0 + M],
                            r_t[:, hl * W + X0 : hl * W + X0 + NMM],
                            start=True, stop=True,
                        )
                    dst_ab = AP(rect_g.tensor, rect_g.offset + hl * ABW,
                                [[rp, 128], [176, 2], [1, 176]])
                    src_ab = AP(g_ps.tensor, g_ps.offset, [[pp, 128], [256, 2], [1, 176]])
                    dst_c = rect_g[:, CBASE + hl * CW_ : CBASE + (hl + 1) * CW_]
                    src_c = g_ps[:, 512 : 512 + 112]
                    if hl % 2 == 0:
                        nc.vector.tensor_copy(dst_ab, src_ab)
                        nc.scalar.copy(dst_c, src_c)
                    else:
                        nc.scalar.copy(dst_ab, src_ab)
                        nc.vector.tensor_copy(dst_c, src_c)

                band_g = band_pool.tile([128, 8 * 96], mybir.dt.bfloat16, tag="band")
                nc.gpsimd.local_scatter(
                    band_g[:, : nrows * 96], rect_g[:, : nrows * ABW],
                    idx_t[:, : nrows * ABW],
                    channels=128, num_elems=nrows * 96, num_idxs=nrows * ABW,
                )
                pending.append(("band", gi, band_g))
                pending_q.append(("dump", gi, rect_g))
                while pending_q and pending_q[0][1] <= gi - 4:
                    _emit_store(nc, out2, quads, pending_q.pop(0))
                while pending and pending[0][1] <= gi - STORE_LAG:
                    _emit_store(nc, out2, quads, pending.pop(0))
            for item in pending_q + pending:
                _emit_store(nc, out2, quads, item)
    nc.compile()
    return nc


def _get_nc(_mode=None):
    if "nc" not in _cache:
        _cache["nc"] = _build()
    return _cache["nc"]


def kernel(left_feature, right_feature):
    left_feature = np.asarray(left_feature, dtype=np.float32)
    right_feature = np.asarray(right_feature, dtype=np.float32)
    b, c, h, w = left_feature.shape
    assert (b, c, h, w) == (B, C, H, W)
    nc = _get_nc()
    idx = make_idxs()
    in_maps = []
    for i in range(B):
        lf = (left_feature[i].reshape(C, HW) * np.float32(1.0 / C)).astype(ml_dtypes.bfloat16)
        rf = right_feature[i].reshape(C, HW).astype(ml_dtypes.bfloat16)
        in_maps.append({
            "left": np.ascontiguousarray(lf),
            "right": np.ascontiguousarray(rf),
            "idxs": idx,
        })
    trace = bool(os.environ.get("KERNEL_TRACE"))
    res = run_bass_kernel_spmd(nc, in_maps, core_ids=list(range(B)), trace=trace)
    if trace:
        print("HW exec time:", res.exec_time_ns, "ns")
    outs = []
    a32 = np.arange(32)
    for i in range(B):
        vol = np.empty((D, H, W), dtype=np.float32)
        raw = np.asarray(res.results[i]["out2"]).astype(np.float32)
        for gi, (h0, nr) in enumerate(GROUPS):
            band = raw[OUT2_OFF[gi] : OUT2_OFF[gi + 1]].reshape(128, nr, 2, D)
            v = band.transpose(3, 1, 2, 0)  # [k, hl, ci, a]
            vol[:, h0 : h0 + nr, :256] = v.reshape(D, nr, 256)
        qall = np.asarray(res.results[i]["quads"]).astype(np.float32).reshape(
            len(GROUPS), 2, 32, 8, 80)  # [g, quad, a32, hl, col80]
        for gi, (h0, nr) in enumerate(GROUPS):
            for k in range(D):
                sel = (a32 + k)[None, :, None, None]
                blk = np.take_along_axis(qall[gi], sel, axis=3)[:, :, :, 0]  # [2, 32, 8]
                vol[k, h0 : h0 + nr, 256:] = blk.reshape(64, 8)[:, :nr].T
        outs.append(vol[::-1])  # k = 47 - i
    out = np.stack(outs, axis=0)
    for i in range(1, D):
        out[:, i, :, :i] = 0.0
    return out


if __name__ == "__main__":
    rng = np.random.default_rng(0)
    lf = rng.standard_normal((B, C, H, W), dtype=np.float32)
    rf = rng.standard_normal((B, C, H, W), dtype=np.float32)
    got = kernel(lf, rf)
    for (bb, i, hh, xx) in [(0, 0, 0, 0), (0, 5, 10, 100), (1, 47, 95, 319), (2, 47, 3, 10),
                            (3, 20, 85, 200), (7, 1, 90, 300), (5, 30, 35, 128)]:
        want = float(np.dot(lf[bb, :, hh, xx], rf[bb, :, hh, xx - i]) / C) if xx >= i else 0.0
        print((bb, i, hh, xx), "got", got[bb, i, hh, xx], "want", want)


# revision 39
# speedup vs baseline: 1.0094x; 1.0061x over previous
"""Correlation cost volume kernel for Trainium2 (8 NeuronCores, batch-parallel).

cost[b, i, h, x] = mean_c left[b,c,h,x] * right[b,c,h,x-i], i in [0,48), zero for x < i.

Per core (one batch element):
  Inputs are host-cast to bf16, left pre-scaled by 1/128 (exact power of two),
  so no on-device scaling is needed and all DMA traffic is halved.
  For each h row and x-chunk (M=128/128/64): PSUM G[a, j] = sum_c
  lscaled[c, X0+a] * right[c, X0-47+j]. Right is loaded contiguously with
  slack; out-of-range columns read garbage that only reaches the x < i
  triangle, which the host masks to zero.
  PSUM tile [128, 1024] (2 banks; chunk slots at {0,256,512} so no matmul
  crosses a bank). Two DVE/ACT copies per h row cast to bf16 into the group
  rect tile, hl-major row blocks of 464 (A 176 | B 176 | C 112).
  Shear band[a, (hl*3+ci)*48 + k] = G[a, a+k]:
   - scatter groups: one gpsimd local_scatter per group (per-partition
     indices; invalid lanes zeroed) + one full-rate contiguous store.
     hl-major makes a 4-row group's index table a prefix of the full one,
     so the first/last groups are 4-row halves (earlier Pool start/finish).
   - dump groups: 10 quad-block DMAs store the 80-wide diagonal quarters;
     the host extracts the diagonals (no Pool time).
  Shear-store DMAs are emitted a few groups late on nc.sync so their waits
  are satisfied at issue time and never stall a sequencer.
  Host untangles layouts -> (i=47-k, h, x), flips i, zeroes x < i.
"""
import os

import numpy as np
import ml_dtypes

import concourse.bacc as bacc
import concourse.mybir as mybir
import concourse.tile as tile
from concourse.ap import AP
from concourse.bass_utils import run_bass_kernel_spmd

B, C, H, W = 8, 128, 96, 320
D = 48  # disparities
HW = H * W
CHUNKS = [(0, 128, 176), (128, 128, 176), (256, 64, 112)]  # (X0, M, NMM)
ABW = 352  # AB row block: A 176 | B 176 (scatter region)
CW_ = 112  # C row block width (host-shear region)
CBASE = 8 * ABW  # C region offset inside rect (2816)
RECW = 8 * ABW + 8 * CW_  # 3712
RW = 47 + 8 * W + 48  # right tile width incl. slack (2655)
# (h0, nrows, kind): 'S' = gpsimd local_scatter shear, 'D' = quad dump (host shear)
GROUPS = [(0, 4), (4, 4), (8, 8), (16, 8), (24, 8), (32, 8), (40, 8),
          (48, 8), (56, 8), (64, 8), (72, 8), (80, 8), (88, 4), (92, 4)]
OUT2_OFF = np.cumsum([0] + [128 * nr * 96 for h0, nr in GROUPS]).tolist()
QB = 32 * 8 * 80  # one C quad block
QTOT = 2 * QB  # per group (quads 0,1 cover partitions a<64)
STORE_LAG = 7

_cache = {}


def make_idxs():
    """idx[a, hl*352 + ci*176 + col] = (hl*2+ci)*48 + (col-a), col in [a, a+48)."""
    idx = np.full((128, CBASE), -1, dtype=np.int16)
    a = np.arange(128)
    for hl in range(8):
        for ci in range(2):
            s = hl * 2 + ci
            for k in range(D):
                idx[a, hl * ABW + ci * 176 + a + k] = s * D + k
    return idx


def _emit_store(nc, out2, quads, item):
    kind, gi, tile_ = item
    h0, nrows = GROUPS[gi]
    if kind == "band":
        dst = AP(out2.tensor, out2.offset + OUT2_OFF[gi],
                 [[nrows * 96, 128], [1, nrows * 96]])
        nc.sync.dma_start(out=dst, in_=tile_[:, : nrows * 96])
        return
    rp = tile_.ap[0][0]
    qbase = quads.offset + gi * QTOT
    for q in range(2):  # C quads: rows [32q,32q+32), cols [32q,32q+80)
        src = AP(tile_.tensor, tile_.offset + 32 * q * rp + CBASE + 32 * q,
                 [[rp, 32], [CW_, 8], [1, 80]])
        dst = AP(quads.tensor, qbase + q * QB, [[8 * 80, 32], [80, 8], [1, 80]])
        nc.sync.dma_start(out=dst, in_=src)


def _build():
    nc = bacc.Bacc("TRN2", target_bir_lowering=False, debug=False, num_devices=8)
    lr = nc.dram_tensor("lr", [C, 2 * HW], mybir.dt.bfloat16, kind="ExternalInput").ap()
    idxs_in = nc.dram_tensor("idxs", [128, CBASE], mybir.dt.int16, kind="ExternalInput").ap()
    out2 = nc.dram_tensor("out2", [OUT2_OFF[-1]], mybir.dt.bfloat16,
                          kind="ExternalOutput").ap()
    quads = nc.dram_tensor("quads", [len(GROUPS) * QTOT], mybir.dt.bfloat16,
                           kind="ExternalOutput").ap()

    with tile.TileContext(nc) as tc:
        with (
            tc.tile_pool(name="io", bufs=8) as io_pool,
            tc.tile_pool(name="rect", bufs=8) as rect_pool,
            tc.tile_pool(name="band", bufs=8) as band_pool,
            tc.tile_pool(name="const", bufs=1) as const_pool,
            tc.tile_pool(name="ps", bufs=4, space="PSUM") as ps_pool,
        ):
            idx_t = const_pool.tile([128, CBASE], mybir.dt.int16)
            pending = []
            pending_q = []

            for gi, (h0, nrows) in enumerate(GROUPS):
                lr_t = io_pool.tile([C, 8 * W + RW], mybir.dt.bfloat16, tag="lrt")
                lp_ = lr_t.ap[0][0]
                ROFF = 8 * W + 47  # r starts here inside lr_t
                dst = AP(lr_t.tensor, lr_t.offset,
                         [[lp_, C], [ROFF, 2], [1, nrows * W]])
                srcin = AP(lr.tensor, lr.offset + h0 * W,
                           [[2 * HW, C], [HW, 2], [1, nrows * W]])
                nc.sync.dma_start(out=dst, in_=srcin)
                # idx table loaded in prefix pieces so early small groups
                # can scatter before the whole table arrives
                if gi == 0:
                    nc.sync.dma_start(out=idx_t[:, : 4 * ABW],
                                      in_=idxs_in[:, : 4 * ABW])
                elif gi == 1:
                    nc.sync.dma_start(out=idx_t[:, 4 * ABW :],
                                      in_=idxs_in[:, 4 * ABW :])

                rect_g = rect_pool.tile([128, RECW], mybir.dt.bfloat16, tag="rect")
                rp = rect_g.ap[0][0]
                for hl in range(nrows):
                    # 2 PSUM banks; chunk slots at {0,256,512}: no bank crossing.
                    g_ps = ps_pool.tile([128, 1024], mybir.dt.float32, tag="gps")
                    pp = g_ps.ap[0][0]
                    for ci, (X0, M, NMM) in enumerate(CHUNKS):
                        nc.tensor.matmul(
                            g_ps[:M, ci * 256 : ci * 256 + NMM],
                            lr_t[:, hl * W + X0 : hl * W + X0 + M],
                            lr_t[:, ROFF - 47 + hl * W + X0 :
                                 ROFF - 47 + hl * W + X0 + NMM],
                            start=True, stop=True,
                        )
                    dst_ab = AP(rect_g.tensor, rect_g.offset + hl * ABW,
                                [[rp, 128], [176, 2], [1, 176]])
                    src_ab = AP(g_ps.tensor, g_ps.offset, [[pp, 128], [256, 2], [1, 176]])
                    dst_c = rect_g[:, CBASE + hl * CW_ : CBASE + (hl + 1) * CW_]
                    src_c = g_ps[:, 512 : 512 + 112]
                    if hl % 2 == 0:
                        nc.vector.tensor_copy(dst_ab, src_ab)
                        nc.scalar.copy(dst_c, src_c)
                    else:
                        nc.scalar.copy(dst_ab, src_ab)
                        nc.vector.tensor_copy(dst_c, src_c)

                band_g = band_pool.tile([128, 8 * 96], mybir.dt.bfloat16, tag="band")
                nc.gpsimd.local_scatter(
                    band_g[:, : nrows * 96], rect_g[:, : nrows * ABW],
                    idx_t[:, : nrows * ABW],
                    channels=128, num_elems=nrows * 96, num_idxs=nrows * ABW,
                )
                pending.append(("band", gi, band_g))
                pending_q.append(("dump", gi, rect_g))
                while pending_q and pending_q[0][1] <= gi - 4:
                    _emit_store(nc, out2, quads, pending_q.pop(0))
                while pending and pending[0][1] <= gi - STORE_LAG:
                    _emit_store(nc, out2, quads, pending.pop(0))
            for item in pending_q + pending:
                _emit_store(nc, out2, quads, item)
    nc.compile()
    return nc


def _get_nc(_mode=None):
    if "nc" not in _cache:
        _cache["nc"] = _build()
    return _cache["nc"]


def kernel(left_feature, right_feature):
    left_feature = np.asarray(left_feature, dtype=np.float32)
    right_feature = np.asarray(right_feature, dtype=np.float32)
    b, c, h, w = left_feature.shape
    assert (b, c, h, w) == (B, C, H, W)
    nc = _get_nc()
    idx = make_idxs()
    in_maps = []
    for i in range(B):
        lf = (left_feature[i].reshape(C, HW) * np.float32(1.0 / C)).astype(ml_dtypes.bfloat16)
        rf = right_feature[i].reshape(C, HW).astype(ml_dtypes.bfloat16)
        in_maps.append({
            "lr": np.ascontiguousarray(np.concatenate([lf, rf], axis=1)),
            "idxs": idx,
        })
    trace = bool(os.environ.get("KERNEL_TRACE"))
    res = run_bass_kernel_spmd(nc, in_maps, core_ids=list(range(B)), trace=trace)
    if trace:
        print("HW exec time:", res.exec_time_ns, "ns")
    outs = []
    a32 = np.arange(32)
    for i in range(B):
        vol = np.empty((D, H, W), dtype=np.float32)
        raw = np.asarray(res.results[i]["out2"]).astype(np.float32)
        for gi, (h0, nr) in enumerate(GROUPS):
            band = raw[OUT2_OFF[gi] : OUT2_OFF[gi + 1]].reshape(128, nr, 2, D)
            v = band.transpose(3, 1, 2, 0)  # [k, hl, ci, a]
            vol[:, h0 : h0 + nr, :256] = v.reshape(D, nr, 256)
        qall = np.asarray(res.results[i]["quads"]).astype(np.float32).reshape(
            len(GROUPS), 2, 32, 8, 80)  # [g, quad, a32, hl, col80]
        for gi, (h0, nr) in enumerate(GROUPS):
            for k in range(D):
                sel = (a32 + k)[None, :, None, None]
                blk = np.take_along_axis(qall[gi], sel, axis=3)[:, :, :, 0]  # [2, 32, 8]
                vol[k, h0 : h0 + nr, 256:] = blk.reshape(64, 8)[:, :nr].T
        outs.append(vol[::-1])  # k = 47 - i
    out = np.stack(outs, axis=0)
    for i in range(1, D):
        out[:, i, :, :i] = 0.0
    return out


if __name__ == "__main__":
    rng = np.random.default_rng(0)
    lf = rng.standard_normal((B, C, H, W), dtype=np.float32)
    rf = rng.standard_normal((B, C, H, W), dtype=np.float32)
    got = kernel(lf, rf)
    for (bb, i, hh, xx) in [(0, 0, 0, 0), (0, 5, 10, 100), (1, 47, 95, 319), (2, 47, 3, 10),
                            (3, 20, 85, 200), (7, 1, 90, 300), (5, 30, 35, 128)]:
        want = float(np.dot(lf[bb, :, hh, xx], rf[bb, :, hh, xx - i]) / C) if xx >= i else 0.0
        print((bb, i, hh, xx), "got", got[bb, i, hh, xx], "want", want)


# revision 40
# speedup vs baseline: 1.0117x; 1.0023x over previous
"""Correlation cost volume kernel for Trainium2 (8 NeuronCores, batch-parallel).

cost[b, i, h, x] = mean_c left[b,c,h,x] * right[b,c,h,x-i], i in [0,48), zero for x < i.

Per core (one batch element):
  Inputs are host-cast to bf16, left pre-scaled by 1/128 (exact power of two),
  so no on-device scaling is needed and all DMA traffic is halved.
  For each h row and x-chunk (M=128/128/64): PSUM G[a, j] = sum_c
  lscaled[c, X0+a] * right[c, X0-47+j]. Right is loaded contiguously with
  slack; out-of-range columns read garbage that only reaches the x < i
  triangle, which the host masks to zero.
  PSUM tile [128, 1024] (2 banks; chunk slots at {0,256,512} so no matmul
  crosses a bank). Two DVE/ACT copies per h row cast to bf16 into the group
  rect tile, hl-major row blocks of 464 (A 176 | B 176 | C 112).
  Shear band[a, (hl*3+ci)*48 + k] = G[a, a+k]:
   - scatter groups: one gpsimd local_scatter per group (per-partition
     indices; invalid lanes zeroed) + one full-rate contiguous store.
     hl-major makes a 4-row group's index table a prefix of the full one,
     so the first/last groups are 4-row halves (earlier Pool start/finish).
   - dump groups: 10 quad-block DMAs store the 80-wide diagonal quarters;
     the host extracts the diagonals (no Pool time).
  Shear-store DMAs are emitted a few groups late on nc.sync so their waits
  are satisfied at issue time and never stall a sequencer.
  Host untangles layouts -> (i=47-k, h, x), flips i, zeroes x < i.
"""
import os

import numpy as np
import ml_dtypes

import concourse.bacc as bacc
import concourse.mybir as mybir
import concourse.tile as tile
from concourse.ap import AP
from concourse.bass_utils import run_bass_kernel_spmd

B, C, H, W = 8, 128, 96, 320
D = 48  # disparities
HW = H * W
CHUNKS = [(0, 128, 176), (128, 128, 176), (256, 64, 112)]  # (X0, M, NMM)
ABW = 352  # AB row block: A 176 | B 176 (scatter region)
CW_ = 112  # C row block width (host-shear region)
CBASE = 8 * ABW  # C region offset inside rect (2816)
RECW = 8 * ABW + 8 * CW_  # 3712
RW = 47 + 8 * W + 48  # right tile width incl. slack (2655)
# (h0, nrows, kind): 'S' = gpsimd local_scatter shear, 'D' = quad dump (host shear)
GROUPS = [(0, 4), (4, 4), (8, 8), (16, 8), (24, 8), (32, 8), (40, 8),
          (48, 8), (56, 8), (64, 8), (72, 8), (80, 8), (88, 4), (92, 4)]
OUT2_OFF = np.cumsum([0] + [128 * nr * 96 for h0, nr in GROUPS]).tolist()
QB = 32 * 8 * 80  # one C quad block
QTOT = 2 * QB  # per group (quads 0,1 cover partitions a<64)
STORE_LAG = 6

_cache = {}


def make_idxs():
    """idx[a, hl*352 + ci*176 + col] = (hl*2+ci)*48 + (col-a), col in [a, a+48)."""
    idx = np.full((128, CBASE), -1, dtype=np.int16)
    a = np.arange(128)
    for hl in range(8):
        for ci in range(2):
            s = hl * 2 + ci
            for k in range(D):
                idx[a, hl * ABW + ci * 176 + a + k] = s * D + k
    return idx


def _emit_store(nc, out2, quads, item):
    kind, gi, tile_ = item
    h0, nrows = GROUPS[gi]
    if kind == "band":
        dst = AP(out2.tensor, out2.offset + OUT2_OFF[gi],
                 [[nrows * 96, 128], [1, nrows * 96]])
        nc.sync.dma_start(out=dst, in_=tile_[:, : nrows * 96])
        return
    rp = tile_.ap[0][0]
    qbase = quads.offset + gi * QTOT
    for q in range(2):  # C quads: rows [32q,32q+32), cols [32q,32q+80)
        src = AP(tile_.tensor, tile_.offset + 32 * q * rp + CBASE + 32 * q,
                 [[rp, 32], [CW_, 8], [1, 80]])
        dst = AP(quads.tensor, qbase + q * QB, [[8 * 80, 32], [80, 8], [1, 80]])
        nc.sync.dma_start(out=dst, in_=src)


def _build():
    nc = bacc.Bacc("TRN2", target_bir_lowering=False, debug=False, num_devices=8)
    lr = nc.dram_tensor("lr", [C, 2 * HW], mybir.dt.bfloat16, kind="ExternalInput").ap()
    idxs_in = nc.dram_tensor("idxs", [128, CBASE], mybir.dt.int16, kind="ExternalInput").ap()
    out2 = nc.dram_tensor("out2", [OUT2_OFF[-1]], mybir.dt.bfloat16,
                          kind="ExternalOutput").ap()
    quads = nc.dram_tensor("quads", [len(GROUPS) * QTOT], mybir.dt.bfloat16,
                           kind="ExternalOutput").ap()

    with tile.TileContext(nc) as tc:
        with (
            tc.tile_pool(name="io", bufs=8) as io_pool,
            tc.tile_pool(name="rect", bufs=8) as rect_pool,
            tc.tile_pool(name="band", bufs=8) as band_pool,
            tc.tile_pool(name="const", bufs=1) as const_pool,
            tc.tile_pool(name="ps", bufs=4, space="PSUM") as ps_pool,
        ):
            idx_t = const_pool.tile([128, CBASE], mybir.dt.int16)
            pending = []
            pending_q = []

            for gi, (h0, nrows) in enumerate(GROUPS):
                lr_t = io_pool.tile([C, 8 * W + RW], mybir.dt.bfloat16, tag="lrt")
                lp_ = lr_t.ap[0][0]
                ROFF = 8 * W + 47  # r starts here inside lr_t
                dst = AP(lr_t.tensor, lr_t.offset,
                         [[lp_, C], [ROFF, 2], [1, nrows * W]])
                srcin = AP(lr.tensor, lr.offset + h0 * W,
                           [[2 * HW, C], [HW, 2], [1, nrows * W]])
                nc.sync.dma_start(out=dst, in_=srcin)
                # idx table loaded in prefix pieces so early small groups
                # can scatter before the whole table arrives
                if gi == 0:
                    nc.sync.dma_start(out=idx_t[:, : 4 * ABW],
                                      in_=idxs_in[:, : 4 * ABW])
                elif gi == 1:
                    nc.sync.dma_start(out=idx_t[:, 4 * ABW :],
                                      in_=idxs_in[:, 4 * ABW :])

                rect_g = rect_pool.tile([128, RECW], mybir.dt.bfloat16, tag="rect")
                rp = rect_g.ap[0][0]
                for hl in range(nrows):
                    # 2 PSUM banks; chunk slots at {0,256,512}: no bank crossing.
                    g_ps = ps_pool.tile([128, 1024], mybir.dt.float32, tag="gps")
                    pp = g_ps.ap[0][0]
                    for ci, (X0, M, NMM) in enumerate(CHUNKS):
                        nc.tensor.matmul(
                            g_ps[:M, ci * 256 : ci * 256 + NMM],
                            lr_t[:, hl * W + X0 : hl * W + X0 + M],
                            lr_t[:, ROFF - 47 + hl * W + X0 :
                                 ROFF - 47 + hl * W + X0 + NMM],
                            start=True, stop=True,
                        )
                    dst_ab = AP(rect_g.tensor, rect_g.offset + hl * ABW,
                                [[rp, 128], [176, 2], [1, 176]])
                    src_ab = AP(g_ps.tensor, g_ps.offset, [[pp, 128], [256, 2], [1, 176]])
                    dst_c = rect_g[:, CBASE + hl * CW_ : CBASE + (hl + 1) * CW_]
                    src_c = g_ps[:, 512 : 512 + 112]
                    if hl % 2 == 0:
                        nc.vector.tensor_copy(dst_ab, src_ab)
                        nc.scalar.copy(dst_c, src_c)
                    else:
                        nc.scalar.copy(dst_ab, src_ab)
                        nc.vector.tensor_copy(dst_c, src_c)

                band_g = band_pool.tile([128, 8 * 96], mybir.dt.bfloat16, tag="band")
                nc.gpsimd.local_scatter(
                    band_g[:, : nrows * 96], rect_g[:, : nrows * ABW],
                    idx_t[:, : nrows * ABW],
                    channels=128, num_elems=nrows * 96, num_idxs=nrows * ABW,
                )
                pending.append(("band", gi, band_g))
                pending_q.append(("dump", gi, rect_g))
                while pending_q and pending_q[0][1] <= gi - 4:
                    _emit_store(nc, out2, quads, pending_q.pop(0))
                while pending and pending[0][1] <= gi - STORE_LAG:
                    _emit_store(nc, out2, quads, pending.pop(0))
            for item in pending_q + pending:
                _emit_store(nc, out2, quads, item)
    nc.compile()
    return nc


def _get_nc(_mode=None):
    if "nc" not in _cache:
        _cache["nc"] = _build()
    return _cache["nc"]


def kernel(left_feature, right_feature):
    left_feature = np.asarray(left_feature, dtype=np.float32)
    right_feature = np.asarray(right_feature, dtype=np.float32)
    b, c, h, w = left_feature.shape
    assert (b, c, h, w) == (B, C, H, W)
    nc = _get_nc()
    idx = make_idxs()
    in_maps = []
    for i in range(B):
        lf = (left_feature[i].reshape(C, HW) * np.float32(1.0 / C)).astype(ml_dtypes.bfloat16)
        rf = right_feature[i].reshape(C, HW).astype(ml_dtypes.bfloat16)
        in_maps.append({
            "lr": np.ascontiguousarray(np.concatenate([lf, rf], axis=1)),
            "idxs": idx,
        })
    trace = bool(os.environ.get("KERNEL_TRACE"))
    res = run_bass_kernel_spmd(nc, in_maps, core_ids=list(range(B)), trace=trace)
    if trace:
        print("HW exec time:", res.exec_time_ns, "ns")
    outs = []
    a32 = np.arange(32)
    for i in range(B):
        vol = np.empty((D, H, W), dtype=np.float32)
        raw = np.asarray(res.results[i]["out2"]).astype(np.float32)
        for gi, (h0, nr) in enumerate(GROUPS):
            band = raw[OUT2_OFF[gi] : OUT2_OFF[gi + 1]].reshape(128, nr, 2, D)
            v = band.transpose(3, 1, 2, 0)  # [k, hl, ci, a]
            vol[:, h0 : h0 + nr, :256] = v.reshape(D, nr, 256)
        qall = np.asarray(res.results[i]["quads"]).astype(np.float32).reshape(
            len(GROUPS), 2, 32, 8, 80)  # [g, quad, a32, hl, col80]
        for gi, (h0, nr) in enumerate(GROUPS):
            for k in range(D):
                sel = (a32 + k)[None, :, None, None]
                blk = np.take_along_axis(qall[gi], sel, axis=3)[:, :, :, 0]  # [2, 32, 8]
                vol[k, h0 : h0 + nr, 256:] = blk.reshape(64, 8)[:, :nr].T
        outs.append(vol[::-1])  # k = 47 - i
    out = np.stack(outs, axis=0)
    for i in range(1, D):
        out[:, i, :, :i] = 0.0
    return out


if __name__ == "__main__":
    rng = np.random.default_rng(0)
    lf = rng.standard_normal((B, C, H, W), dtype=np.float32)
    rf = rng.standard_normal((B, C, H, W), dtype=np.float32)
    got = kernel(lf, rf)
    for (bb, i, hh, xx) in [(0, 0, 0, 0), (0, 5, 10, 100), (1, 47, 95, 319), (2, 47, 3, 10),
                            (3, 20, 85, 200), (7, 1, 90, 300), (5, 30, 35, 128)]:
        want = float(np.dot(lf[bb, :, hh, xx], rf[bb, :, hh, xx - i]) / C) if xx >= i else 0.0
        print((bb, i, hh, xx), "got", got[bb, i, hh, xx], "want", want)
